# revision 12
# baseline (speedup 1.0000x reference)
"""EdgeDecoder kernel for 8 Trainium2 NeuronCores.

Math: out[e] = dot(x_src[i0[e]], w_src) + dot(x_dst[i1[e]], w_dst) + bias.
Rewritten as per-node scores s[n] = x_src[n]@w_src + bias, d[n] = x_dst[n]@w_dst,
then out[e] = s[i0[e]] + d[i1[e]].

Device pipeline (nodes sharded 8-way, edges sharded 8-way):
  Launch 1 (scores): each core loads its 12500-node slice of x_src/x_dst as
    bf16 [h=128, n] (host-transposed), runs 98 matmuls per side
    (lhsT = x^T chunk [128h,128n] stationary, rhs = [w_src|w_dst] [128h,2])
    and drains the per-node scores to a tiny bf16 [128, 2, 196] table.
    DMA is ~6.4 MB/core (the x slice, read exactly once) + 0.1 MB out --
    no per-edge tensors touch the device in this launch at all.
  Host gathers the score tables per edge (pure permutation/cast, no
    arithmetic) into the two bf16 halves of every edge.
  Launch 2 (edge add): each core streams its 250k edges' two halves
    [128, 2, 1954] bf16, adds them on DVE, stores [128, 1954] bf16.
"""

import numpy as np
import ml_dtypes

BF16 = ml_dtypes.bfloat16

N_NODES = 100000
HIDDEN = 128
N_EDGES = 2000000
N_CORES = 8
NS = N_NODES // N_CORES         # 12500 nodes per core
CH = 98                         # matmul chunks of 128 nodes per side (12544)
NPAD = CH * 128                 # padded nodes per core per side
# x-load DMA group sizes (chunks); descending so the post-stream tail only
# waits on a small final transfer
XGRP = [40, 40, 17, 1]
NB = 8                          # PSUM banks; bank k holds a contiguous run
# bank boundaries (13,26,40,53,66,80,94,98) align with x-group boundaries
# (40,80,97,98) so at most one bank drains after the last x bytes land
BCOL = [13, 13, 14, 13, 13, 14, 14, 4]  # chunks per bank (sum = 98)
OFFB = np.concatenate([[0], np.cumsum(BCOL)])[:NB]
# matmul chunk m -> (bank, col) with contiguous runs: banks drain early
B_OF = np.repeat(np.arange(NB), BCOL)
C_OF = np.concatenate([np.arange(c) for c in BCOL])
# score columns stay in chunk order (bank k drains to cols OFFB[k]:...)
M2C = np.arange(CH)
PER = N_EDGES // N_CORES        # 250000 edges per launch-2 core
COLS = (PER + 127) // 128       # 1954
E_OUT = COLS * 128              # 250112 padded launch-2 edges per core

_CACHE = {}


def _mybir():
    import concourse.mybir as mybir
    return mybir


def _build_launch1(reps=1, staggered=False):
    from contextlib import ExitStack
    import concourse.bacc as bacc
    import concourse.tile as tile
    mybir = _mybir()
    f32 = mybir.dt.float32
    bf16 = mybir.dt.bfloat16

    nc = bacc.Bacc("TRN2", debug=False, num_devices=N_CORES)
    xs = nc.dram_tensor("xs", [128, CH, 128], bf16, kind="ExternalInput")
    xd = nc.dram_tensor("xd", [128, CH, 128], bf16, kind="ExternalInput")
    wv = nc.dram_tensor("wv", [128, 2], bf16, kind="ExternalInput")
    biasr = nc.dram_tensor("biasr", [128, 1], f32, kind="ExternalInput")
    sc = nc.dram_tensor("sc", [128, 2, 2 * CH], bf16, kind="ExternalOutput")

    with tile.TileContext(nc) as tc:
        with tc.tile_pool(name="const", bufs=1) as cp, \
             tc.tile_pool(name="xload", bufs=6) as xp, \
             tc.tile_pool(name="work", bufs=2) as wp, \
             tc.tile_pool(name="psum", bufs=1, space="PSUM") as pp:

            wv_t = cp.tile([128, 2], bf16)
            nc.sync.dma_start(out=wv_t[:], in_=wv.ap()[:, :])
            bias_t = cp.tile([128, 1], f32, name="bias_t")
            nc.sync.dma_start(out=bias_t[:], in_=biasr.ap()[:, :])

            _loop = ExitStack()
            if reps > 1:
                _loop.enter_context(
                    tc.For_i(0, reps, 1,
                             hint_engines=(mybir.EngineType.PE,),
                             staggered_reset=staggered))

            # per-node scores for both sides; column sidx*CH + m of row
            # 'wcol' holds w[wcol] . x_side[node m*128+p]
            w = wp.tile([128, 2, 2 * CH], bf16, name="w_t", tag="w")

            def side(x, sidx, nm, use_bias):
                # chunk m writes PSUM bank B_OF[m], col C_OF[m]: banks fill
                # in contiguous runs, so each bank drains (and its score
                # slice DMAs out) while later x groups are still streaming.
                pst = [pp.tile([128, 2, BCOL[k]], f32, name=f"ps_{nm}{k}",
                               tag=f"ps{k}") for k in range(NB)]

                def drain(k):
                    # drain (+ bias for the src side) in one DVE pass
                    # (f32 psum -> bf16); DVE is otherwise idle and its
                    # SEQ never queues behind the out-DMA dispatches
                    o0 = sidx * CH + int(OFFB[k])
                    if use_bias:
                        nc.vector.tensor_scalar_add(
                            out=w[:, :, o0:o0 + BCOL[k]],
                            in0=pst[k][:, :, :],
                            scalar1=bias_t[:, :])
                    else:
                        nc.vector.tensor_copy(
                            out=w[:, :, o0:o0 + BCOL[k]],
                            in_=pst[k][:, :, :])

                cut = int(OFFB[NB - 1])
                c0 = 0
                for gi, g in enumerate(XGRP):
                    c1 = c0 + g
                    xt = xp.tile([128, XGRP[0], 128], bf16,
                                 name=f"xt_{nm}{c0}", tag="xt")
                    nc.sync.dma_start(
                        out=xt[:, :g, :],
                        in_=x.ap()[:, c0:c1, :])
                    for j in range(g):
                        m = c0 + j
                        nc.tensor.matmul(
                            pst[B_OF[m]][:, :, C_OF[m]:C_OF[m] + 1],
                            xt[:, j, :],
                            wv_t[:, :])
                        if C_OF[m] == BCOL[B_OF[m]] - 1:
                            drain(int(B_OF[m]))
                            if B_OF[m] == NB - 2:
                                # banks 0-6 done: ship them while bank 7's
                                # x group is still streaming
                                nc.scalar.dma_start(
                                    out=sc.ap()[:, :,
                                                sidx * CH:sidx * CH + cut],
                                    in_=w[:, :, sidx * CH:sidx * CH + cut])
                    c0 = c1
                # the tiny bank-7 slice rides the idle SP ring at the end
                nc.sync.dma_start(
                    out=sc.ap()[:, :, sidx * CH + cut:(sidx + 1) * CH],
                    in_=w[:, :, sidx * CH + cut:(sidx + 1) * CH])

            side(xs, 0, "s", True)
            side(xd, 1, "d", False)
            _loop.close()

    nc.compile()
    return nc


def _build_launch2(reps=1, staggered=False):
    from contextlib import ExitStack
    import concourse.bacc as bacc
    import concourse.tile as tile
    mybir = _mybir()
    bf16 = mybir.dt.bfloat16

    nc = bacc.Bacc("TRN2", debug=False, num_devices=N_CORES)
    a01 = nc.dram_tensor("a01", [128, 2, COLS], bf16, kind="ExternalInput")
    o = nc.dram_tensor("o", [128, COLS], bf16, kind="ExternalOutput")
    with tile.TileContext(nc) as tc:
        with tc.tile_pool(name="io", bufs=3) as io:
            _loop = ExitStack()
            if reps > 1:
                _loop.enter_context(
                    tc.For_i(0, reps, 1, staggered_reset=staggered))
            c0 = 0
            for step in (904, 904, 146):
                c1 = min(c0 + step, COLS)
                t0 = io.tile([128, 2, 904], bf16, name=f"t0_{c0}", tag="t0")
                to = io.tile([128, 904], bf16, name=f"to_{c0}", tag="to")
                nc.sync.dma_start(out=t0[:, :, :c1 - c0],
                                  in_=a01.ap()[:, :, c0:c1])
                nc.vector.tensor_tensor(out=to[:, :c1 - c0],
                                        in0=t0[:, 0, :c1 - c0],
                                        in1=t0[:, 1, :c1 - c0],
                                        op=mybir.AluOpType.add)
                nc.scalar.dma_start(out=o.ap()[:, c0:c1], in_=to[:, :c1 - c0])
                c0 = c1
            _loop.close()
    nc.compile()
    return nc


def _stage_x(x):
    """x slice [NS, H] f32 -> bf16 [h=128, CH, 128]: chunk m column j holds
    x of local node m*128+j (zero-padded past NS)."""
    xt = np.zeros((128, NPAD), BF16)
    xt[:, :NS] = x.astype(BF16).T
    return np.ascontiguousarray(xt.reshape(128, CH, 128))


def _decode_scores(sc_all):
    """Per-core device score tables [N_CORES][128, 2, 2*CH] -> full-table
    (s, d) f32-indexable bf16 arrays of length N_NODES."""
    s = np.empty(N_CORES * NPAD, BF16)
    d = np.empty(N_CORES * NPAD, BF16)
    for c in range(N_CORES):
        # node m*128+p of this core sits at [p, wcol, sidx*CH + M2C[m]]
        s[c * NPAD:(c + 1) * NPAD] = \
            sc_all[c][:, 0, M2C].T.reshape(-1)
        d[c * NPAD:(c + 1) * NPAD] = \
            sc_all[c][:, 1, CH + M2C].T.reshape(-1)
    return s, d


def _run_with_retry(nc, in_maps, attempts=3):
    """The axon-tunneled devices occasionally report a transient
    NRT_EXEC_UNIT_UNRECOVERABLE; a spaced retry usually succeeds."""
    import time
    from concourse import bass_utils
    last = None
    for k in range(attempts):
        try:
            return bass_utils.run_bass_kernel_spmd(
                nc, in_maps, core_ids=list(range(N_CORES)))
        except Exception as e:  # noqa: BLE001 - device transient
            last = e
            time.sleep(3.0 * (k + 1))
    raise last


def kernel(x_src, x_dst, edge_label_index, weight, bias):
    x_src = np.ascontiguousarray(np.asarray(x_src, dtype=np.float32))
    x_dst = np.ascontiguousarray(np.asarray(x_dst, dtype=np.float32))
    idx = np.asarray(edge_label_index)
    i0 = idx[0].astype(np.int64)
    i1 = idx[1].astype(np.int64)
    wgt = np.asarray(weight, dtype=np.float32)
    b = np.asarray(bias, dtype=np.float32)

    if "l1" not in _CACHE:
        _CACHE["l1"] = _build_launch1()
    if "l2" not in _CACHE:
        _CACHE["l2"] = _build_launch2()
    nc1, nc2 = _CACHE["l1"], _CACHE["l2"]

    # w staged on partitions (K = h), one column per side
    wv = np.zeros((128, 2), BF16)
    wv[:, 0] = wgt[0, :HIDDEN].astype(BF16)
    wv[:, 1] = wgt[0, HIDDEN:].astype(BF16)

    in_maps1 = []
    for c in range(N_CORES):
        in_maps1.append({
            "xs": _stage_x(x_src[c * NS:(c + 1) * NS]),
            "xd": _stage_x(x_dst[c * NS:(c + 1) * NS]),
            "wv": wv,
            "biasr": np.full((128, 1), b[0], np.float32),
        })
    res1 = _run_with_retry(nc1, in_maps1)
    s_tab, d_tab = _decode_scores(
        [res1.results[c]["sc"] for c in range(N_CORES)])

    # gather score halves per edge (host permutation only); a node's slot in
    # the concatenated table is (n // NS) * NPAD + (n % NS)
    v0 = s_tab[(i0 // NS) * NPAD + i0 % NS]
    v1 = d_tab[(i1 // NS) * NPAD + i1 % NS]
    in_maps2 = []
    for c in range(N_CORES):
        a = np.zeros((2, E_OUT), BF16)
        a[0, :PER] = v0[c * PER:(c + 1) * PER]
        a[1, :PER] = v1[c * PER:(c + 1) * PER]
        in_maps2.append({
            "a01": np.ascontiguousarray(
                a.reshape(2, 128, COLS).transpose(1, 0, 2)),
        })
    res2 = _run_with_retry(nc2, in_maps2)

    out = np.empty(N_EDGES, np.float32)
    for c in range(N_CORES):
        out[c * PER:(c + 1) * PER] = \
            res2.results[c]["o"].reshape(-1)[:PER].astype(np.float32)
    return out.reshape(N_EDGES, 1)


# revision 17
# speedup vs baseline: 1.5606x; 1.5606x over previous
"""EdgeDecoder kernel for 8 Trainium2 NeuronCores.

Math: out[e] = dot(x_src[i0[e]], w_src) + dot(x_dst[i1[e]], w_dst) + bias.
Rewritten as per-node scores s[n] = x_src[n]@w_src + bias, d[n] = x_dst[n]@w_dst,
then out[e] = s[i0[e]] + d[i1[e]].

Device pipeline (nodes sharded 8-way, edges sharded 8-way):
  Launch 1 (scores): each core loads its 12500-node slice of x_src/x_dst as
    bf16 [h=128, n] (host-transposed), runs 98 matmuls per side
    (lhsT = x^T chunk [128h,128n] stationary, rhs = [w_src|w_dst] [128h,2])
    and drains the per-node scores to a tiny bf16 [128, 2, 196] table.
    DMA is ~6.4 MB/core (the x slice, read exactly once) + 0.1 MB out --
    no per-edge tensors touch the device in this launch at all.
  Host gathers the score tables per edge (pure permutation/cast, no
    arithmetic) into the two bf16 halves of every edge.
  Launch 2 (edge add): each core streams its 250k edges' two halves
    [128, 2, 1954] bf16, adds them on DVE, stores [128, 1954] bf16.
"""

import numpy as np
import ml_dtypes

BF16 = ml_dtypes.bfloat16

N_NODES = 100000
HIDDEN = 128
N_EDGES = 2000000
N_CORES = 8
NS = N_NODES // N_CORES         # 12500 nodes per core
CH = 98                         # matmul chunks of 128 nodes per side (12544)
NPAD = CH * 128                 # padded nodes per core per side
# x-load DMA group sizes (chunks); descending so the post-stream tail only
# waits on a small final transfer
XGRP = [40, 40, 17, 1]
NB = 8                          # PSUM banks; bank k holds a contiguous run
# bank boundaries (13,26,40,53,66,80,94,98) align with x-group boundaries
# (40,80,97,98) so at most one bank drains after the last x bytes land
BCOL = [13, 13, 14, 13, 13, 14, 14, 4]  # chunks per bank (sum = 98)
OFFB = np.concatenate([[0], np.cumsum(BCOL)])[:NB]
# matmul chunk m -> (bank, col) with contiguous runs: banks drain early
B_OF = np.repeat(np.arange(NB), BCOL)
C_OF = np.concatenate([np.arange(c) for c in BCOL])
# score columns stay in chunk order (bank k drains to cols OFFB[k]:...)
M2C = np.arange(CH)
PER = N_EDGES // N_CORES        # 250000 edges per launch-2 core
COLS = (PER + 127) // 128       # 1954
E_OUT = COLS * 128              # 250112 padded launch-2 edges per core

_CACHE = {}


def _mybir():
    import concourse.mybir as mybir
    return mybir


def _build_launch1(reps=1, staggered=False):
    from contextlib import ExitStack
    import concourse.bacc as bacc
    import concourse.tile as tile
    mybir = _mybir()
    f32 = mybir.dt.float32
    bf16 = mybir.dt.bfloat16

    nc = bacc.Bacc("TRN2", debug=False, num_devices=N_CORES)
    xs = nc.dram_tensor("xs", [128, CH, 128], bf16, kind="ExternalInput")
    xd = nc.dram_tensor("xd", [128, CH, 128], bf16, kind="ExternalInput")
    wv = nc.dram_tensor("wv", [128, 2], bf16, kind="ExternalInput")
    biasr = nc.dram_tensor("biasr", [128, 1], f32, kind="ExternalInput")
    sc = nc.dram_tensor("sc", [128, 2 * CH], bf16, kind="ExternalOutput")

    with tile.TileContext(nc) as tc:
        with tc.tile_pool(name="const", bufs=1) as cp, \
             tc.tile_pool(name="xload", bufs=6) as xp, \
             tc.tile_pool(name="work", bufs=2) as wp, \
             tc.tile_pool(name="psum", bufs=1, space="PSUM") as pp:

            wv_t = cp.tile([128, 2], bf16)
            nc.sync.dma_start(out=wv_t[:], in_=wv.ap()[:, :])
            bias_t = cp.tile([128, 1], f32, name="bias_t")
            nc.sync.dma_start(out=bias_t[:], in_=biasr.ap()[:, :])

            _loop = ExitStack()
            if reps > 1:
                _loop.enter_context(
                    tc.For_i(0, reps, 1,
                             hint_engines=(mybir.EngineType.PE,),
                             staggered_reset=staggered))

            # per-node scores for both sides; column sidx*CH + m holds
            # w[sidx] . x_side[node m*128+p]
            w = wp.tile([128, 2 * CH], bf16, name="w_t", tag="w")

            def side(x, sidx, nm, use_bias):
                # chunk m writes PSUM bank B_OF[m], col C_OF[m]: banks fill
                # in contiguous runs, so each bank drains (and its score
                # slice DMAs out) while later x groups are still streaming.
                pst = [pp.tile([128, BCOL[k]], f32, name=f"ps_{nm}{k}",
                               tag=f"ps{k}") for k in range(NB)]

                def drain(k):
                    # drain (+ bias for the src side) in one DVE pass
                    # (f32 psum -> bf16); DVE is otherwise idle and its
                    # SEQ never queues behind the out-DMA dispatches
                    o0 = sidx * CH + int(OFFB[k])
                    if use_bias:
                        nc.vector.tensor_scalar_add(
                            out=w[:, o0:o0 + BCOL[k]],
                            in0=pst[k][:, :],
                            scalar1=bias_t[:, :])
                    else:
                        nc.vector.tensor_copy(
                            out=w[:, o0:o0 + BCOL[k]],
                            in_=pst[k][:, :])

                cut = int(OFFB[NB - 1])
                c0 = 0
                for gi, g in enumerate(XGRP):
                    c1 = c0 + g
                    xt = xp.tile([128, XGRP[0], 128], bf16,
                                 name=f"xt_{nm}{c0}", tag="xt")
                    # alternate the two HWDGE rings (SP / ACT) so one ring's
                    # per-DMA bookkeeping overlaps the other's data stream
                    eng = nc.scalar if (gi + sidx) % 2 else nc.sync
                    eng.dma_start(
                        out=xt[:, :g, :],
                        in_=x.ap()[:, c0:c1, :])
                    for j in range(g):
                        m = c0 + j
                        nc.tensor.matmul(
                            pst[B_OF[m]][:, C_OF[m]:C_OF[m] + 1],
                            xt[:, j, :],
                            wv_t[:, sidx:sidx + 1])
                        if C_OF[m] == BCOL[B_OF[m]] - 1:
                            drain(int(B_OF[m]))
                            if B_OF[m] == NB - 2:
                                # banks 0-6 done: ship them while bank 7's
                                # x group is still streaming
                                nc.scalar.dma_start(
                                    out=sc.ap()[:,
                                                sidx * CH:sidx * CH + cut],
                                    in_=w[:, sidx * CH:sidx * CH + cut])
                    c0 = c1
                # the tiny bank-7 slice rides the idle SP ring at the end
                nc.sync.dma_start(
                    out=sc.ap()[:, sidx * CH + cut:(sidx + 1) * CH],
                    in_=w[:, sidx * CH + cut:(sidx + 1) * CH])

            side(xs, 0, "s", True)
            side(xd, 1, "d", False)
            _loop.close()

    nc.compile()
    return nc


def _build_launch2(reps=1, staggered=False):
    from contextlib import ExitStack
    import concourse.bacc as bacc
    import concourse.tile as tile
    mybir = _mybir()
    bf16 = mybir.dt.bfloat16

    nc = bacc.Bacc("TRN2", debug=False, num_devices=N_CORES)
    a01 = nc.dram_tensor("a01", [128, 2, COLS], bf16, kind="ExternalInput")
    o = nc.dram_tensor("o", [128, COLS], bf16, kind="ExternalOutput")
    with tile.TileContext(nc) as tc:
        with tc.tile_pool(name="io", bufs=3) as io:
            _loop = ExitStack()
            if reps > 1:
                _loop.enter_context(
                    tc.For_i(0, reps, 1, staggered_reset=staggered))
            c0 = 0
            for ci, step in enumerate((904, 904, 146)):
                c1 = min(c0 + step, COLS)
                t0 = io.tile([128, 2, 904], bf16, name=f"t0_{c0}", tag="t0")
                to = io.tile([128, 904], bf16, name=f"to_{c0}", tag="to")
                # in/out DMAs alternate between the two HWDGE rings
                ein = nc.scalar if ci % 2 else nc.sync
                eout = nc.sync if ci % 2 else nc.scalar
                ein.dma_start(out=t0[:, :, :c1 - c0],
                              in_=a01.ap()[:, :, c0:c1])
                nc.vector.tensor_tensor(out=to[:, :c1 - c0],
                                        in0=t0[:, 0, :c1 - c0],
                                        in1=t0[:, 1, :c1 - c0],
                                        op=mybir.AluOpType.add)
                eout.dma_start(out=o.ap()[:, c0:c1], in_=to[:, :c1 - c0])
                c0 = c1
            _loop.close()
    nc.compile()
    return nc


def _stage_x(x):
    """x slice [NS, H] f32 -> bf16 [h=128, CH, 128]: chunk m column j holds
    x of local node m*128+j (zero-padded past NS)."""
    xt = np.zeros((128, NPAD), BF16)
    xt[:, :NS] = x.astype(BF16).T
    return np.ascontiguousarray(xt.reshape(128, CH, 128))


def _decode_scores(sc_all):
    """Per-core device score tables [N_CORES][128, 2*CH] -> full-table
    (s, d) bf16 arrays of length N_CORES*NPAD (node n of core c at
    c*NPAD + n)."""
    s = np.empty(N_CORES * NPAD, BF16)
    d = np.empty(N_CORES * NPAD, BF16)
    for c in range(N_CORES):
        # node m*128+p of this core sits at [p, sidx*CH + m]
        s[c * NPAD:(c + 1) * NPAD] = sc_all[c][:, :CH].T.reshape(-1)
        d[c * NPAD:(c + 1) * NPAD] = sc_all[c][:, CH:].T.reshape(-1)
    return s, d


def _run_with_retry(nc, in_maps, attempts=3):
    """The axon-tunneled devices occasionally report a transient
    NRT_EXEC_UNIT_UNRECOVERABLE; a spaced retry usually succeeds."""
    import time
    from concourse import bass_utils
    last = None
    for k in range(attempts):
        try:
            return bass_utils.run_bass_kernel_spmd(
                nc, in_maps, core_ids=list(range(N_CORES)))
        except Exception as e:  # noqa: BLE001 - device transient
            last = e
            time.sleep(3.0 * (k + 1))
    raise last


def kernel(x_src, x_dst, edge_label_index, weight, bias):
    x_src = np.ascontiguousarray(np.asarray(x_src, dtype=np.float32))
    x_dst = np.ascontiguousarray(np.asarray(x_dst, dtype=np.float32))
    idx = np.asarray(edge_label_index)
    i0 = idx[0].astype(np.int64)
    i1 = idx[1].astype(np.int64)
    wgt = np.asarray(weight, dtype=np.float32)
    b = np.asarray(bias, dtype=np.float32)

    if "l1" not in _CACHE:
        _CACHE["l1"] = _build_launch1()
    if "l2" not in _CACHE:
        _CACHE["l2"] = _build_launch2()
    nc1, nc2 = _CACHE["l1"], _CACHE["l2"]

    # w staged on partitions (K = h), one column per side
    wv = np.zeros((128, 2), BF16)
    wv[:, 0] = wgt[0, :HIDDEN].astype(BF16)
    wv[:, 1] = wgt[0, HIDDEN:].astype(BF16)

    in_maps1 = []
    for c in range(N_CORES):
        in_maps1.append({
            "xs": _stage_x(x_src[c * NS:(c + 1) * NS]),
            "xd": _stage_x(x_dst[c * NS:(c + 1) * NS]),
            "wv": wv,
            "biasr": np.full((128, 1), b[0], np.float32),
        })
    res1 = _run_with_retry(nc1, in_maps1)
    s_tab, d_tab = _decode_scores(
        [res1.results[c]["sc"] for c in range(N_CORES)])

    # gather score halves per edge (host permutation only); a node's slot in
    # the concatenated table is (n // NS) * NPAD + (n % NS)
    v0 = s_tab[(i0 // NS) * NPAD + i0 % NS]
    v1 = d_tab[(i1 // NS) * NPAD + i1 % NS]
    in_maps2 = []
    for c in range(N_CORES):
        a = np.zeros((2, E_OUT), BF16)
        a[0, :PER] = v0[c * PER:(c + 1) * PER]
        a[1, :PER] = v1[c * PER:(c + 1) * PER]
        in_maps2.append({
            "a01": np.ascontiguousarray(
                a.reshape(2, 128, COLS).transpose(1, 0, 2)),
        })
    res2 = _run_with_retry(nc2, in_maps2)

    out = np.empty(N_EDGES, np.float32)
    for c in range(N_CORES):
        out[c * PER:(c + 1) * PER] = \
            res2.results[c]["o"].reshape(-1)[:PER].astype(np.float32)
    return out.reshape(N_EDGES, 1)


# revision 18
# speedup vs baseline: 1.7073x; 1.0940x over previous
"""EdgeDecoder kernel for 8 Trainium2 NeuronCores.

Math: out[e] = dot(x_src[i0[e]], w_src) + dot(x_dst[i1[e]], w_dst) + bias.
Rewritten as per-node scores s[n] = x_src[n]@w_src + bias, d[n] = x_dst[n]@w_dst,
then out[e] = s[i0[e]] + d[i1[e]].

Device pipeline (nodes sharded 8-way, edges sharded 8-way):
  Launch 1 (scores): each core loads its 12500-node slice of x_src/x_dst as
    bf16 [h=128, n] (host-transposed), runs 98 matmuls per side
    (lhsT = x^T chunk [128h,128n] stationary, rhs = [w_src|w_dst] [128h,2])
    and drains the per-node scores to a tiny bf16 [128, 2, 196] table.
    DMA is ~6.4 MB/core (the x slice, read exactly once) + 0.1 MB out --
    no per-edge tensors touch the device in this launch at all.
  Host gathers the score tables per edge (pure permutation/cast, no
    arithmetic) into the two bf16 halves of every edge.
  Launch 2 (edge add): each core streams its 250k edges' two halves
    [128, 2, 1954] bf16, adds them on DVE, stores [128, 1954] bf16.
"""

import numpy as np
import ml_dtypes

BF16 = ml_dtypes.bfloat16

N_NODES = 100000
HIDDEN = 128
N_EDGES = 2000000
N_CORES = 8
NS = N_NODES // N_CORES         # 12500 nodes per core
CH = 98                         # matmul chunks of 128 nodes per side (12544)
NPAD = CH * 128                 # padded nodes per core per side
# x-load DMA group sizes (chunks); descending so the post-stream tail only
# waits on a small final transfer
XGRP = [49, 48, 1]
NB = 8                          # PSUM banks; bank k holds a contiguous run
# bank boundaries (13,26,39,49,61,73,86,98) align with x-group boundaries
# (49,97,98) so at most one bank drains after the last x bytes land
BCOL = [13, 13, 13, 10, 12, 12, 13, 12]  # chunks per bank (sum = 98)
OFFB = np.concatenate([[0], np.cumsum(BCOL)])[:NB]
# matmul chunk m -> (bank, col) with contiguous runs: banks drain early
B_OF = np.repeat(np.arange(NB), BCOL)
C_OF = np.concatenate([np.arange(c) for c in BCOL])
# score columns stay in chunk order (bank k drains to cols OFFB[k]:...)
M2C = np.arange(CH)
PER = N_EDGES // N_CORES        # 250000 edges per launch-2 core
COLS = (PER + 127) // 128       # 1954
E_OUT = COLS * 128              # 250112 padded launch-2 edges per core

_CACHE = {}


def _mybir():
    import concourse.mybir as mybir
    return mybir


def _build_launch1(reps=1, staggered=False):
    from contextlib import ExitStack
    import concourse.bacc as bacc
    import concourse.tile as tile
    mybir = _mybir()
    f32 = mybir.dt.float32
    bf16 = mybir.dt.bfloat16

    nc = bacc.Bacc("TRN2", debug=False, num_devices=N_CORES)
    xs = nc.dram_tensor("xs", [128, CH, 128], bf16, kind="ExternalInput")
    xd = nc.dram_tensor("xd", [128, CH, 128], bf16, kind="ExternalInput")
    wv = nc.dram_tensor("wv", [128, 2], bf16, kind="ExternalInput")
    biasr = nc.dram_tensor("biasr", [128, 1], f32, kind="ExternalInput")
    sc = nc.dram_tensor("sc", [128, 2 * CH], bf16, kind="ExternalOutput")

    with tile.TileContext(nc) as tc:
        with tc.tile_pool(name="const", bufs=1) as cp, \
             tc.tile_pool(name="xload", bufs=6) as xp, \
             tc.tile_pool(name="work", bufs=2) as wp, \
             tc.tile_pool(name="psum", bufs=1, space="PSUM") as pp:

            wv_t = cp.tile([128, 2], bf16)
            nc.sync.dma_start(out=wv_t[:], in_=wv.ap()[:, :])
            bias_t = cp.tile([128, 1], f32, name="bias_t")
            nc.sync.dma_start(out=bias_t[:], in_=biasr.ap()[:, :])

            _loop = ExitStack()
            if reps > 1:
                _loop.enter_context(
                    tc.For_i(0, reps, 1,
                             hint_engines=(mybir.EngineType.PE,),
                             staggered_reset=staggered))

            # per-node scores for both sides; column sidx*CH + m holds
            # w[sidx] . x_side[node m*128+p]
            w = wp.tile([128, 2 * CH], bf16, name="w_t", tag="w")

            def side(x, sidx, nm, use_bias):
                # chunk m writes PSUM bank B_OF[m], col C_OF[m]: banks fill
                # in contiguous runs, so each bank drains (and its score
                # slice DMAs out) while later x groups are still streaming.
                pst = [pp.tile([128, BCOL[k]], f32, name=f"ps_{nm}{k}",
                               tag=f"ps{k}") for k in range(NB)]

                def drain(k):
                    # drain (+ bias for the src side) in one DVE pass
                    # (f32 psum -> bf16); DVE is otherwise idle and its
                    # SEQ never queues behind the out-DMA dispatches
                    o0 = sidx * CH + int(OFFB[k])
                    if use_bias:
                        nc.vector.tensor_scalar_add(
                            out=w[:, o0:o0 + BCOL[k]],
                            in0=pst[k][:, :],
                            scalar1=bias_t[:, :])
                    else:
                        nc.vector.tensor_copy(
                            out=w[:, o0:o0 + BCOL[k]],
                            in_=pst[k][:, :])

                cut = int(OFFB[NB - 1])
                c0 = 0
                for gi, g in enumerate(XGRP):
                    c1 = c0 + g
                    xt = xp.tile([128, XGRP[0], 128], bf16,
                                 name=f"xt_{nm}{c0}", tag="xt")
                    # alternate the two HWDGE rings (SP / ACT) so one ring's
                    # per-DMA bookkeeping overlaps the other's data stream
                    eng = nc.scalar if (gi + sidx) % 2 else nc.sync
                    eng.dma_start(
                        out=xt[:, :g, :],
                        in_=x.ap()[:, c0:c1, :])
                    for j in range(g):
                        m = c0 + j
                        nc.tensor.matmul(
                            pst[B_OF[m]][:, C_OF[m]:C_OF[m] + 1],
                            xt[:, j, :],
                            wv_t[:, sidx:sidx + 1])
                        if C_OF[m] == BCOL[B_OF[m]] - 1:
                            drain(int(B_OF[m]))
                            if B_OF[m] == NB - 2:
                                # banks 0-6 done: ship them while bank 7's
                                # x group is still streaming
                                nc.scalar.dma_start(
                                    out=sc.ap()[:,
                                                sidx * CH:sidx * CH + cut],
                                    in_=w[:, sidx * CH:sidx * CH + cut])
                    c0 = c1
                # the tiny bank-7 slice rides the idle SP ring at the end
                nc.sync.dma_start(
                    out=sc.ap()[:, sidx * CH + cut:(sidx + 1) * CH],
                    in_=w[:, sidx * CH + cut:(sidx + 1) * CH])

            side(xs, 0, "s", True)
            side(xd, 1, "d", False)
            _loop.close()

    nc.compile()
    return nc


def _build_launch2(reps=1, staggered=False):
    from contextlib import ExitStack
    import concourse.bacc as bacc
    import concourse.tile as tile
    mybir = _mybir()
    bf16 = mybir.dt.bfloat16

    nc = bacc.Bacc("TRN2", debug=False, num_devices=N_CORES)
    a01 = nc.dram_tensor("a01", [128, 2, COLS], bf16, kind="ExternalInput")
    o = nc.dram_tensor("o", [128, COLS], bf16, kind="ExternalOutput")
    with tile.TileContext(nc) as tc:
        with tc.tile_pool(name="io", bufs=3) as io:
            _loop = ExitStack()
            if reps > 1:
                _loop.enter_context(
                    tc.For_i(0, reps, 1, staggered_reset=staggered))
            c0 = 0
            for ci, step in enumerate((904, 904, 146)):
                c1 = min(c0 + step, COLS)
                t0 = io.tile([128, 2, 904], bf16, name=f"t0_{c0}", tag="t0")
                to = io.tile([128, 904], bf16, name=f"to_{c0}", tag="to")
                # in/out DMAs alternate between the two HWDGE rings
                ein = nc.scalar if ci % 2 else nc.sync
                eout = nc.sync if ci % 2 else nc.scalar
                ein.dma_start(out=t0[:, :, :c1 - c0],
                              in_=a01.ap()[:, :, c0:c1])
                nc.vector.tensor_tensor(out=to[:, :c1 - c0],
                                        in0=t0[:, 0, :c1 - c0],
                                        in1=t0[:, 1, :c1 - c0],
                                        op=mybir.AluOpType.add)
                eout.dma_start(out=o.ap()[:, c0:c1], in_=to[:, :c1 - c0])
                c0 = c1
            _loop.close()
    nc.compile()
    return nc


def _stage_x(x):
    """x slice [NS, H] f32 -> bf16 [h=128, CH, 128]: chunk m column j holds
    x of local node m*128+j (zero-padded past NS)."""
    xt = np.zeros((128, NPAD), BF16)
    xt[:, :NS] = x.astype(BF16).T
    return np.ascontiguousarray(xt.reshape(128, CH, 128))


def _decode_scores(sc_all):
    """Per-core device score tables [N_CORES][128, 2*CH] -> full-table
    (s, d) bf16 arrays of length N_CORES*NPAD (node n of core c at
    c*NPAD + n)."""
    s = np.empty(N_CORES * NPAD, BF16)
    d = np.empty(N_CORES * NPAD, BF16)
    for c in range(N_CORES):
        # node m*128+p of this core sits at [p, sidx*CH + m]
        s[c * NPAD:(c + 1) * NPAD] = sc_all[c][:, :CH].T.reshape(-1)
        d[c * NPAD:(c + 1) * NPAD] = sc_all[c][:, CH:].T.reshape(-1)
    return s, d


def _run_with_retry(nc, in_maps, attempts=3):
    """The axon-tunneled devices occasionally report a transient
    NRT_EXEC_UNIT_UNRECOVERABLE; a spaced retry usually succeeds."""
    import time
    from concourse import bass_utils
    last = None
    for k in range(attempts):
        try:
            return bass_utils.run_bass_kernel_spmd(
                nc, in_maps, core_ids=list(range(N_CORES)))
        except Exception as e:  # noqa: BLE001 - device transient
            last = e
            time.sleep(3.0 * (k + 1))
    raise last


def kernel(x_src, x_dst, edge_label_index, weight, bias):
    x_src = np.ascontiguousarray(np.asarray(x_src, dtype=np.float32))
    x_dst = np.ascontiguousarray(np.asarray(x_dst, dtype=np.float32))
    idx = np.asarray(edge_label_index)
    i0 = idx[0].astype(np.int64)
    i1 = idx[1].astype(np.int64)
    wgt = np.asarray(weight, dtype=np.float32)
    b = np.asarray(bias, dtype=np.float32)

    if "l1" not in _CACHE:
        _CACHE["l1"] = _build_launch1()
    if "l2" not in _CACHE:
        _CACHE["l2"] = _build_launch2()
    nc1, nc2 = _CACHE["l1"], _CACHE["l2"]

    # w staged on partitions (K = h), one column per side
    wv = np.zeros((128, 2), BF16)
    wv[:, 0] = wgt[0, :HIDDEN].astype(BF16)
    wv[:, 1] = wgt[0, HIDDEN:].astype(BF16)

    in_maps1 = []
    for c in range(N_CORES):
        in_maps1.append({
            "xs": _stage_x(x_src[c * NS:(c + 1) * NS]),
            "xd": _stage_x(x_dst[c * NS:(c + 1) * NS]),
            "wv": wv,
            "biasr": np.full((128, 1), b[0], np.float32),
        })
    res1 = _run_with_retry(nc1, in_maps1)
    s_tab, d_tab = _decode_scores(
        [res1.results[c]["sc"] for c in range(N_CORES)])

    # gather score halves per edge (host permutation only); a node's slot in
    # the concatenated table is (n // NS) * NPAD + (n % NS)
    v0 = s_tab[(i0 // NS) * NPAD + i0 % NS]
    v1 = d_tab[(i1 // NS) * NPAD + i1 % NS]
    in_maps2 = []
    for c in range(N_CORES):
        a = np.zeros((2, E_OUT), BF16)
        a[0, :PER] = v0[c * PER:(c + 1) * PER]
        a[1, :PER] = v1[c * PER:(c + 1) * PER]
        in_maps2.append({
            "a01": np.ascontiguousarray(
                a.reshape(2, 128, COLS).transpose(1, 0, 2)),
        })
    res2 = _run_with_retry(nc2, in_maps2)

    out = np.empty(N_EDGES, np.float32)
    for c in range(N_CORES):
        out[c * PER:(c + 1) * PER] = \
            res2.results[c]["o"].reshape(-1)[:PER].astype(np.float32)
    return out.reshape(N_EDGES, 1)
